# revision 35
# baseline (speedup 1.0000x reference)
"""Trainium2 Bass kernel for nn_AdjunctiveClassifier (segment_reduce).

MLP (1024->512 relu ->2) over 131072 patches + attention-gated evidence +
per-bag segment sum.  Data-parallel over 8 NeuronCores: each core gets a
contiguous shard of 16384 patches (= 16 bags of 1024), weights replicated.

Device layout: everything is kept transposed ([dim, patch]) so that
 - matmul1: h^T[hid,patch] = W1^T-tiles (stationary, natural W1 slices)
            @ X^T-tiles (moving) , accumulated over 8 K-tiles in PSUM
 - matmul2: logits^T[2,patch] = W2-tiles (stationary, natural W2 slices)
            @ h^T (moving), accumulated over 4 hid-tiles
 - epilogue is elementwise on [2, patch] rows, bag sums are free-axis
   reductions over 1024-patch segments.
X^T is produced on the host (free w.r.t. HW exec time); evidence outputs
come back as [2, shard] rows and are transposed back on the host.
The matmul path runs in bfloat16 (1 moving col/cycle on the PE; fp32
would be 4x slower, float32r measured ~1.3 cyc/col due to SBUF
bandwidth); PSUM accumulation and the epilogue stay fp32.  A short
dummy-matmul warmup flips the PE HAM clock gate to full speed while the
first tiles are still in flight.
"""

import sys

import ml_dtypes
import numpy as np

if "/opt/trn_rl_repo" not in sys.path:  # harness safety: concourse import path
    try:
        import concourse  # noqa: F401
    except ImportError:
        sys.path.insert(0, "/opt/trn_rl_repo")

import concourse.bacc as bacc
import concourse.bass as bass
import concourse.mybir as mybir
from concourse import tile
from concourse.bass_utils import run_bass_kernel_spmd

N_PATCHES = 131072
N_BAGS = 128
IN_DIM = 1024
HID_DIM = 512
OUT_DIM = 2
N_CORES = 8
SHARD = N_PATCHES // N_CORES          # 16384 patches per core
BAGS_PER_CORE = N_BAGS // N_CORES     # 16
BAG_SIZE = N_PATCHES // N_BAGS        # 1024
GROUP = BAG_SIZE                      # one bag per group
HALF = 512                            # PSUM-bank / fp32-moving limit
K_TILES = IN_DIM // 128               # 8
H_TILES = HID_DIM // 128              # 4

DT = mybir.dt.float32
DTR = mybir.dt.bfloat16
AFT = mybir.ActivationFunctionType


def build_nc(shard: int = SHARD):
    """Build + compile the per-core Bass graph for a `shard`-patch shard."""
    n_groups = shard // GROUP
    nc = bacc.Bacc("TRN2", target_bir_lowering=False, debug=False,
                   num_devices=N_CORES)

    n_halves = shard // HALF
    xt_e = nc.declare_dram_parameter("xt", [n_halves, 128, K_TILES, HALF], DTR, isOutput=False)
    att_e = nc.declare_dram_parameter("att4", [4, shard], DT, isOutput=False)
    w1_e = nc.declare_dram_parameter("w1", [128, K_TILES, HID_DIM], DTR, isOutput=False)
    w2_e = nc.declare_dram_parameter("w2", [128, H_TILES, OUT_DIM], DTR, isOutput=False)
    b1_e = nc.declare_dram_parameter("b1c", [128, H_TILES], DT, isOutput=False)
    b2_e = nc.declare_dram_parameter("b2c", [OUT_DIM, 2], DT, isOutput=False)
    pos_e = nc.declare_dram_parameter("pos_t", [OUT_DIM, shard], DT, isOutput=True)
    neg_e = nc.declare_dram_parameter("neg_t", [OUT_DIM, shard], DT, isOutput=True)
    net_e = nc.declare_dram_parameter("net_t", [OUT_DIM, shard], DT, isOutput=True)
    bag_e = nc.declare_dram_parameter("bag_t", [OUT_DIM, n_groups], DT, isOutput=True)

    with tile.TileContext(nc) as tc:
        with (
            tc.tile_pool(name="const", bufs=1) as constp,
            tc.tile_pool(name="xtp", bufs=5) as xtp,
            tc.tile_pool(name="hp", bufs=2) as hp,
            tc.tile_pool(name="epp", bufs=2) as epp,
            tc.tile_pool(name="hps", bufs=6, space=bass.MemorySpace.PSUM) as hpsp,
            tc.tile_pool(name="lps", bufs=2, space=bass.MemorySpace.PSUM) as lpsp,
        ):
            w1_sb = constp.tile([128, K_TILES, HID_DIM], DTR)
            nc.sync.dma_start(w1_sb[:], w1_e.ap())
            w2_sb = constp.tile([128, H_TILES, OUT_DIM], DTR)
            nc.sync.dma_start(w2_sb[:], w2_e.ap())
            b1_sb = constp.tile([128, H_TILES], DT)
            nc.sync.dma_start(b1_sb[:], b1_e.ap())
            b2_sb = constp.tile([OUT_DIM, 2], DT)
            nc.sync.dma_start(b2_sb[:], b2_e.ap())
            bag_sb = constp.tile([OUT_DIM, n_groups], DT)
            bagh_sb = constp.tile([OUT_DIM, 2 * n_groups], DT)

            # PE warmup: dummy matmuls flip the HAM clock gate to 8/8
            # while the first xt tiles are still in flight.
            warm_sb = constp.tile([128, HALF], DTR)
            nc.gpsimd.memset(warm_sb[:], 0.0)
            wps = hpsp.tile([128, HALF], DT, tag="hps")
            for _ in range(24):
                nc.tensor.matmul(wps[:], warm_sb[:, 0:128], warm_sb[:],
                                 start=True, stop=True)

            nhalf = GROUP // HALF
            for gp in range(0, n_groups, 2):
                gpair = [gp, gp + 1]
                h_byg = {}
                for g in gpair:
                    xt_ts = []
                    for half in range(nhalf):
                        hidx = g * nhalf + half
                        xt_t = xtp.tile([128, K_TILES, HALF], DTR,
                                        tag=f"xt{half}", name=f"xt_{g}_{half}")
                        for kc in range(0, K_TILES, 2):
                            nc.sync.dma_start(xt_t[:, kc:kc + 2, :],
                                              xt_e.ap()[hidx, :, kc:kc + 2, :])
                        xt_ts.append(xt_t)
                    h_all = [[] for _ in range(nhalf)]
                    for hb in range(H_TILES):
                        hpss = [hpsp.tile([128, HALF], DT, tag="hps",
                                          name=f"hps_{g}_{hb}_{i}")
                                for i in range(nhalf)]
                        for k in range(K_TILES):
                            for half in range(nhalf):
                                nc.tensor.matmul(
                                    hpss[half][:],
                                    w1_sb[:, k, hb * 128:(hb + 1) * 128],
                                    xt_ts[half][:, k, :],
                                    start=(k == 0),
                                    stop=(k == K_TILES - 1),
                                )
                        for half in range(nhalf):
                            h_t = hp.tile([128, HALF], DTR,
                                          tag=f"h{hb}x{half}", bufs=3,
                                          name=f"h_{g}_{hb}_{half}")
                            nc.scalar.activation(h_t[:], hpss[half][:], AFT.Relu,
                                                 bias=b1_sb[:, hb:hb + 1])
                            h_all[half].append(h_t)
                    h_byg[g] = h_all
                last_pair = (gp == n_groups - 2)
                for g in gpair:
                  h_all = h_byg[g]
                  for half in range(nhalf):
                    hidx = g * nhalf + half
                    if not last_pair:
                        # 4 concurrent column-tiled matmuls; partials on PSUM
                        # partition pairs 0/32/64/96, summed via a mixed
                        # PSUM/SBUF chain (ACT copy seeds it)
                        lps = lpsp.tile([128, HALF], DT, tag="lps",
                                        name=f"lps_{g}_{half}")
                        for hb in range(H_TILES):
                            nc.tensor.matmul(
                                lps[32 * hb:32 * hb + 2, :],
                                w2_sb[:, hb, :],
                                h_all[half][hb][:],
                                start=True, stop=True,
                                tile_position=(0, 32 * hb),
                            )
                        c32 = epp.tile([OUT_DIM, HALF], DT, tag="c32")
                        s01 = epp.tile([OUT_DIM, HALF], DT, tag="s01")
                        s02 = epp.tile([OUT_DIM, HALF], DT, tag="s02")
                        lsum = epp.tile([OUT_DIM, HALF], DT, tag="lsum")
                        nc.scalar.copy(c32[:], lps[32:34, :])
                        nc.vector.tensor_add(s01[:], lps[0:2, :], c32[:])
                        nc.vector.scalar_tensor_tensor(
                            s02[:], lps[64:66, :], 1.0, s01[:],
                            mybir.AluOpType.mult, mybir.AluOpType.add)
                        nc.vector.scalar_tensor_tensor(
                            lsum[:], lps[96:98, :], 1.0, s02[:],
                            mybir.AluOpType.mult, mybir.AluOpType.add)
                        lsum_ap = lsum[:]
                    else:
                        # final pair: classic accumulating matmul2 — keeps the
                        # PE in 128-col mode and the exit epilogue short
                        lpsf = lpsp.tile([128, HALF], DT, tag="lps",
                                         name=f"lpsn_{g}_{half}")
                        for hb in range(H_TILES):
                            nc.tensor.matmul(
                                lpsf[0:2, :],
                                w2_sb[:, hb, :],
                                h_all[half][hb][:],
                                start=(hb == 0),
                                stop=(hb == H_TILES - 1),
                            )
                        lsum_ap = lpsf[0:2, :]
                    relu_p = epp.tile([OUT_DIM, HALF], DT, tag="relu_p")
                    min_n = epp.tile([OUT_DIM, HALF], DT, tag="min_n")
                    nc.scalar.activation(relu_p[:], lsum_ap, AFT.Relu,
                                         bias=b2_sb[:, 0:1], scale=1.0)
                    nc.vector.tensor_scalar(min_n[:], lsum_ap, b2_sb[:, 0:1], 0.0,
                                            mybir.AluOpType.add,
                                            mybir.AluOpType.min)

                    hsl = slice(hidx * HALF, (hidx + 1) * HALF)
                    att_p = xtp.tile([OUT_DIM, HALF], DT, tag="attp")
                    att_n = xtp.tile([OUT_DIM, HALF], DT, tag="attn")
                    nc.gpsimd.dma_start(att_p[:], att_e.ap()[0:2, hsl])
                    nc.gpsimd.dma_start(att_n[:], att_e.ap()[2:4, hsl])
                    pos_t = epp.tile([OUT_DIM, HALF], DT, tag="pos")
                    neg_t = epp.tile([OUT_DIM, HALF], DT, tag="neg")
                    net_t = epp.tile([OUT_DIM, HALF], DT, tag="net")
                    nc.vector.tensor_mul(pos_t[:], relu_p[:], att_p[:])
                    nc.vector.tensor_mul(neg_t[:], min_n[:], att_n[:])
                    nc.vector.scalar_tensor_tensor(
                        net_t[:], pos_t[:], 1.0, neg_t[:],
                        mybir.AluOpType.mult, mybir.AluOpType.subtract,
                        accum_out=bagh_sb[:, hidx:hidx + 1])
                    nc.gpsimd.dma_start(pos_e.ap()[:, hsl], pos_t[:])
                    nc.gpsimd.dma_start(neg_e.ap()[:, hsl], neg_t[:])
                    nc.gpsimd.dma_start(net_e.ap()[:, hsl], net_t[:])
            nc.vector.tensor_add(bag_sb[:],
                                 bagh_sb.rearrange("p (g two) -> p g two", two=2)[:, :, 0],
                                 bagh_sb.rearrange("p (g two) -> p g two", two=2)[:, :, 1])
            nc.gpsimd.dma_start(bag_e.ap()[:], bag_sb[:])

    nc.compile()
    return nc


_NC_CACHE: dict = {}


def _get_nc(shard: int = SHARD):
    if shard not in _NC_CACHE:
        _NC_CACHE[shard] = build_nc(shard)
    return _NC_CACHE[shard]


def make_in_maps(features, attention, W1, b1, W2, b2, n_cores=N_CORES):
    shard = features.shape[0] // n_cores
    W1 = np.asarray(W1, dtype=np.float32)
    W2 = np.asarray(W2, dtype=np.float32)
    b1 = np.asarray(b1, dtype=np.float32)
    b2 = np.asarray(b2, dtype=np.float32)
    W1 = np.ascontiguousarray(
        W1.reshape(K_TILES, 128, HID_DIM).transpose(1, 0, 2)
    ).astype(ml_dtypes.bfloat16)
    W2 = np.ascontiguousarray(
        W2.reshape(H_TILES, 128, OUT_DIM).transpose(1, 0, 2)
    ).astype(ml_dtypes.bfloat16)
    b1c = np.ascontiguousarray(b1.reshape(H_TILES, 128).T, dtype=np.float32)
    b2c = np.ascontiguousarray(np.stack([b2, -b2], axis=1), dtype=np.float32)
    in_maps = []
    for i in range(n_cores):
        s = i * shard
        fs = features[s:s + shard]
        at = attention[s:s + shard]
        nh = shard // HALF
        xt = np.ascontiguousarray(
            fs.reshape(nh, HALF, K_TILES, 128).transpose(0, 3, 2, 1)
        ).astype(ml_dtypes.bfloat16)
        a0 = at[:, 0]
        a1 = at[:, 1]
        att4 = np.ascontiguousarray(np.stack([a0, a0, -a1, -a1]), dtype=np.float32)
        in_maps.append({
            "xt": xt, "att4": att4, "w1": W1, "w2": W2, "b1c": b1c, "b2c": b2c,
        })
    return in_maps


def assemble_outputs(results, bag_sizes, n_cores=N_CORES):
    pos = np.concatenate([r["pos_t"].T for r in results], axis=0)
    neg = np.concatenate([r["neg_t"].T for r in results], axis=0)
    net = np.concatenate([r["net_t"].T for r in results], axis=0)
    bag_sizes = np.asarray(bag_sizes)
    if bag_sizes.shape[0] == N_BAGS and np.all(bag_sizes == BAG_SIZE):
        bag = np.concatenate([r["bag_t"].T for r in results], axis=0)
    else:
        # Ragged bags: evidence rows are bag-independent; redo only the
        # tiny [N,2]->[B,2] segment sum on the host.
        starts = np.zeros(bag_sizes.shape[0], dtype=np.int64)
        np.cumsum(bag_sizes[:-1], out=starts[1:])
        bag = np.add.reduceat(net, starts, axis=0).astype(np.float32)
    return bag, pos, neg, net


def kernel(features, attention, bag_sizes, W1, b1, W2, b2):
    features = np.asarray(features, dtype=np.float32)
    attention = np.asarray(attention, dtype=np.float32)
    nc = _get_nc(SHARD)
    in_maps = make_in_maps(features, attention, W1, b1, W2, b2)
    res = run_bass_kernel_spmd(nc, in_maps, list(range(N_CORES)))
    return assemble_outputs(res.results, bag_sizes)


# revision 36
# speedup vs baseline: 1.1716x; 1.1716x over previous
"""Trainium2 Bass kernel for nn_AdjunctiveClassifier (segment_reduce).

MLP (1024->512 relu ->2) over 131072 patches + attention-gated evidence +
per-bag segment sum.  Data-parallel over 8 NeuronCores: each core gets a
contiguous shard of 16384 patches (= 16 bags of 1024), weights replicated.

Device layout: everything is kept transposed ([dim, patch]) so that
 - matmul1: h^T[hid,patch] = W1^T-tiles (stationary, natural W1 slices)
            @ X^T-tiles (moving) , accumulated over 8 K-tiles in PSUM
 - matmul2: logits^T[2,patch] = W2-tiles (stationary, natural W2 slices)
            @ h^T (moving), accumulated over 4 hid-tiles
 - epilogue is elementwise on [2, patch] rows, bag sums are free-axis
   reductions over 1024-patch segments.
X^T is produced on the host (free w.r.t. HW exec time); evidence outputs
come back as [2, shard] rows and are transposed back on the host.
The matmul path runs in bfloat16 (1 moving col/cycle on the PE; fp32
would be 4x slower, float32r measured ~1.3 cyc/col due to SBUF
bandwidth); PSUM accumulation and the epilogue stay fp32.  A short
dummy-matmul warmup flips the PE HAM clock gate to full speed while the
first tiles are still in flight.
"""

import sys

import ml_dtypes
import numpy as np

if "/opt/trn_rl_repo" not in sys.path:  # harness safety: concourse import path
    try:
        import concourse  # noqa: F401
    except ImportError:
        sys.path.insert(0, "/opt/trn_rl_repo")

import concourse.bacc as bacc
import concourse.bass as bass
import concourse.mybir as mybir
from concourse import tile
from concourse.bass_utils import run_bass_kernel_spmd

N_PATCHES = 131072
N_BAGS = 128
IN_DIM = 1024
HID_DIM = 512
OUT_DIM = 2
N_CORES = 8
SHARD = N_PATCHES // N_CORES          # 16384 patches per core
BAGS_PER_CORE = N_BAGS // N_CORES     # 16
BAG_SIZE = N_PATCHES // N_BAGS        # 1024
GROUP = BAG_SIZE                      # one bag per group
HALF = 512                            # PSUM-bank / fp32-moving limit
K_TILES = IN_DIM // 128               # 8
H_TILES = HID_DIM // 128              # 4

DT = mybir.dt.float32
DTR = mybir.dt.bfloat16
AFT = mybir.ActivationFunctionType


def build_nc(shard: int = SHARD):
    """Build + compile the per-core Bass graph for a `shard`-patch shard."""
    n_groups = shard // GROUP
    nc = bacc.Bacc("TRN2", target_bir_lowering=False, debug=False,
                   num_devices=N_CORES)

    n_halves = shard // HALF
    xt_e = nc.declare_dram_parameter("xt", [n_halves, 128, K_TILES, HALF], DTR, isOutput=False)
    att_e = nc.declare_dram_parameter("att4", [4, shard], DT, isOutput=False)
    w1_e = nc.declare_dram_parameter("w1", [128, K_TILES, HID_DIM], DTR, isOutput=False)
    w2_e = nc.declare_dram_parameter("w2", [128, H_TILES, OUT_DIM], DTR, isOutput=False)
    b1_e = nc.declare_dram_parameter("b1c", [128, H_TILES], DT, isOutput=False)
    b2_e = nc.declare_dram_parameter("b2c", [OUT_DIM, 2], DT, isOutput=False)
    pos_e = nc.declare_dram_parameter("pos_t", [OUT_DIM, shard], DT, isOutput=True)
    neg_e = nc.declare_dram_parameter("neg_t", [OUT_DIM, shard], DT, isOutput=True)
    net_e = nc.declare_dram_parameter("net_t", [OUT_DIM, shard], DT, isOutput=True)
    bag_e = nc.declare_dram_parameter("bag_t", [OUT_DIM, n_groups], DT, isOutput=True)

    with tile.TileContext(nc) as tc:
        with (
            tc.tile_pool(name="const", bufs=1) as constp,
            tc.tile_pool(name="xtp", bufs=5) as xtp,
            tc.tile_pool(name="hp", bufs=2) as hp,
            tc.tile_pool(name="epp", bufs=2) as epp,
            tc.tile_pool(name="hps", bufs=6, space=bass.MemorySpace.PSUM) as hpsp,
            tc.tile_pool(name="lps", bufs=2, space=bass.MemorySpace.PSUM) as lpsp,
        ):
            w1_sb = constp.tile([128, K_TILES, HID_DIM], DTR)
            nc.sync.dma_start(w1_sb[:], w1_e.ap())
            w2_sb = constp.tile([128, H_TILES, OUT_DIM], DTR)
            nc.sync.dma_start(w2_sb[:], w2_e.ap())
            b1_sb = constp.tile([128, H_TILES], DT)
            nc.sync.dma_start(b1_sb[:], b1_e.ap())
            b2_sb = constp.tile([OUT_DIM, 2], DT)
            nc.sync.dma_start(b2_sb[:], b2_e.ap())
            bag_sb = constp.tile([OUT_DIM, n_groups], DT)
            bagh_sb = constp.tile([OUT_DIM, 2 * n_groups], DT)

            # PE warmup: dummy matmuls flip the HAM clock gate to 8/8
            # while the first xt tiles are still in flight.
            warm_sb = constp.tile([128, HALF], DTR)
            nc.gpsimd.memset(warm_sb[:], 0.0)
            wps = hpsp.tile([128, HALF], DT, tag="hps")
            for _ in range(24):
                nc.tensor.matmul(wps[:], warm_sb[:, 0:128], warm_sb[:],
                                 start=True, stop=True)

            for g in range(n_groups):
                nhalf = GROUP // HALF
                xt_ts = []
                for half in range(nhalf):
                    hidx = g * nhalf + half
                    xt_t = xtp.tile([128, K_TILES, HALF], DTR, tag=f"xt{half}")
                    for kc in range(0, K_TILES, 2):
                        nc.sync.dma_start(xt_t[:, kc:kc + 2, :],
                                          xt_e.ap()[hidx, :, kc:kc + 2, :])
                    xt_ts.append(xt_t)
                h_all = [[] for _ in range(nhalf)]
                for hb in range(H_TILES):
                    hpss = [hpsp.tile([128, HALF], DT, tag="hps",
                                      name=f"hps_{g}_{hb}_{i}")
                            for i in range(nhalf)]
                    for k in range(K_TILES):
                        for half in range(nhalf):
                            nc.tensor.matmul(
                                hpss[half][:],
                                w1_sb[:, k, hb * 128:(hb + 1) * 128],
                                xt_ts[half][:, k, :],
                                start=(k == 0),
                                stop=(k == K_TILES - 1),
                            )
                    for half in range(nhalf):
                        h_t = hp.tile([128, HALF], DTR, tag=f"h{hb}x{half}")
                        nc.scalar.activation(h_t[:], hpss[half][:], AFT.Relu,
                                             bias=b1_sb[:, hb:hb + 1])
                        h_all[half].append(h_t)
                for half in range(nhalf):
                    hidx = g * nhalf + half
                    lps = lpsp.tile([OUT_DIM, HALF], DT, tag="lps")
                    for hb in range(H_TILES):
                        nc.tensor.matmul(
                            lps[:],
                            w2_sb[:, hb, :],
                            h_all[half][hb][:],
                            start=(hb == 0),
                            stop=(hb == H_TILES - 1),
                        )
                    relu_p = epp.tile([OUT_DIM, HALF], DT, tag="relu_p")
                    min_n = epp.tile([OUT_DIM, HALF], DT, tag="min_n")
                    nc.scalar.activation(relu_p[:], lps[:], AFT.Relu,
                                         bias=b2_sb[:, 0:1], scale=1.0)
                    nc.vector.tensor_scalar(min_n[:], lps[:], b2_sb[:, 0:1], 0.0,
                                            mybir.AluOpType.add,
                                            mybir.AluOpType.min)

                    hsl = slice(hidx * HALF, (hidx + 1) * HALF)
                    att_p = xtp.tile([OUT_DIM, HALF], DT, tag="attp")
                    att_n = xtp.tile([OUT_DIM, HALF], DT, tag="attn")
                    nc.gpsimd.dma_start(att_p[:], att_e.ap()[0:2, hsl])
                    nc.gpsimd.dma_start(att_n[:], att_e.ap()[2:4, hsl])
                    pos_t = epp.tile([OUT_DIM, HALF], DT, tag="pos")
                    neg_t = epp.tile([OUT_DIM, HALF], DT, tag="neg")
                    net_t = epp.tile([OUT_DIM, HALF], DT, tag="net")
                    nc.vector.tensor_mul(pos_t[:], relu_p[:], att_p[:])
                    nc.vector.tensor_mul(neg_t[:], min_n[:], att_n[:])
                    nc.vector.scalar_tensor_tensor(
                        net_t[:], pos_t[:], 1.0, neg_t[:],
                        mybir.AluOpType.mult, mybir.AluOpType.subtract,
                        accum_out=bagh_sb[:, hidx:hidx + 1])
                    nc.gpsimd.dma_start(pos_e.ap()[:, hsl], pos_t[:])
                    nc.gpsimd.dma_start(neg_e.ap()[:, hsl], neg_t[:])
                    nc.gpsimd.dma_start(net_e.ap()[:, hsl], net_t[:])
            nc.vector.tensor_add(bag_sb[:],
                                 bagh_sb.rearrange("p (g two) -> p g two", two=2)[:, :, 0],
                                 bagh_sb.rearrange("p (g two) -> p g two", two=2)[:, :, 1])
            nc.gpsimd.dma_start(bag_e.ap()[:], bag_sb[:])

    nc.compile()
    return nc


_NC_CACHE: dict = {}


def _get_nc(shard: int = SHARD):
    if shard not in _NC_CACHE:
        _NC_CACHE[shard] = build_nc(shard)
    return _NC_CACHE[shard]


def make_in_maps(features, attention, W1, b1, W2, b2, n_cores=N_CORES):
    shard = features.shape[0] // n_cores
    W1 = np.asarray(W1, dtype=np.float32)
    W2 = np.asarray(W2, dtype=np.float32)
    b1 = np.asarray(b1, dtype=np.float32)
    b2 = np.asarray(b2, dtype=np.float32)
    W1 = np.ascontiguousarray(
        W1.reshape(K_TILES, 128, HID_DIM).transpose(1, 0, 2)
    ).astype(ml_dtypes.bfloat16)
    W2 = np.ascontiguousarray(
        W2.reshape(H_TILES, 128, OUT_DIM).transpose(1, 0, 2)
    ).astype(ml_dtypes.bfloat16)
    b1c = np.ascontiguousarray(b1.reshape(H_TILES, 128).T, dtype=np.float32)
    b2c = np.ascontiguousarray(np.stack([b2, -b2], axis=1), dtype=np.float32)
    in_maps = []
    for i in range(n_cores):
        s = i * shard
        fs = features[s:s + shard]
        at = attention[s:s + shard]
        nh = shard // HALF
        xt = np.ascontiguousarray(
            fs.reshape(nh, HALF, K_TILES, 128).transpose(0, 3, 2, 1)
        ).astype(ml_dtypes.bfloat16)
        a0 = at[:, 0]
        a1 = at[:, 1]
        att4 = np.ascontiguousarray(np.stack([a0, a0, -a1, -a1]), dtype=np.float32)
        in_maps.append({
            "xt": xt, "att4": att4, "w1": W1, "w2": W2, "b1c": b1c, "b2c": b2c,
        })
    return in_maps


def assemble_outputs(results, bag_sizes, n_cores=N_CORES):
    pos = np.concatenate([r["pos_t"].T for r in results], axis=0)
    neg = np.concatenate([r["neg_t"].T for r in results], axis=0)
    net = np.concatenate([r["net_t"].T for r in results], axis=0)
    bag_sizes = np.asarray(bag_sizes)
    if bag_sizes.shape[0] == N_BAGS and np.all(bag_sizes == BAG_SIZE):
        bag = np.concatenate([r["bag_t"].T for r in results], axis=0)
    else:
        # Ragged bags: evidence rows are bag-independent; redo only the
        # tiny [N,2]->[B,2] segment sum on the host.
        starts = np.zeros(bag_sizes.shape[0], dtype=np.int64)
        np.cumsum(bag_sizes[:-1], out=starts[1:])
        bag = np.add.reduceat(net, starts, axis=0).astype(np.float32)
    return bag, pos, neg, net


def kernel(features, attention, bag_sizes, W1, b1, W2, b2):
    features = np.asarray(features, dtype=np.float32)
    attention = np.asarray(attention, dtype=np.float32)
    nc = _get_nc(SHARD)
    in_maps = make_in_maps(features, attention, W1, b1, W2, b2)
    res = run_bass_kernel_spmd(nc, in_maps, list(range(N_CORES)))
    return assemble_outputs(res.results, bag_sizes)


# revision 37
# speedup vs baseline: 1.2179x; 1.0395x over previous
"""Trainium2 Bass kernel for nn_AdjunctiveClassifier (segment_reduce).

MLP (1024->512 relu ->2) over 131072 patches + attention-gated evidence +
per-bag segment sum.  Data-parallel over 8 NeuronCores: each core gets a
contiguous shard of 16384 patches (= 16 bags of 1024), weights replicated.

Device layout: everything is kept transposed ([dim, patch]) so that
 - matmul1: h^T[hid,patch] = W1^T-tiles (stationary, natural W1 slices)
            @ X^T-tiles (moving) , accumulated over 8 K-tiles in PSUM
 - matmul2: logits^T[2,patch] = W2-tiles (stationary, natural W2 slices)
            @ h^T (moving), accumulated over 4 hid-tiles
 - epilogue is elementwise on [2, patch] rows, bag sums are free-axis
   reductions over 1024-patch segments.
X^T is produced on the host (free w.r.t. HW exec time); evidence outputs
come back as [2, shard] rows and are transposed back on the host.
The matmul path runs in bfloat16 (1 moving col/cycle on the PE; fp32
would be 4x slower, float32r measured ~1.3 cyc/col due to SBUF
bandwidth); PSUM accumulation and the epilogue stay fp32.  A short
dummy-matmul warmup flips the PE HAM clock gate to full speed while the
first tiles are still in flight.
"""

import sys

import ml_dtypes
import numpy as np

if "/opt/trn_rl_repo" not in sys.path:  # harness safety: concourse import path
    try:
        import concourse  # noqa: F401
    except ImportError:
        sys.path.insert(0, "/opt/trn_rl_repo")

import concourse.bacc as bacc
import concourse.bass as bass
import concourse.mybir as mybir
from concourse import tile
from concourse.bass_utils import run_bass_kernel_spmd

N_PATCHES = 131072
N_BAGS = 128
IN_DIM = 1024
HID_DIM = 512
OUT_DIM = 2
N_CORES = 8
SHARD = N_PATCHES // N_CORES          # 16384 patches per core
BAGS_PER_CORE = N_BAGS // N_CORES     # 16
BAG_SIZE = N_PATCHES // N_BAGS        # 1024
GROUP = BAG_SIZE                      # one bag per group
HALF = 512                            # PSUM-bank / fp32-moving limit
K_TILES = IN_DIM // 128               # 8
H_TILES = HID_DIM // 128              # 4

DT = mybir.dt.float32
DTR = mybir.dt.bfloat16
AFT = mybir.ActivationFunctionType


def build_nc(shard: int = SHARD):
    """Build + compile the per-core Bass graph for a `shard`-patch shard."""
    n_groups = shard // GROUP
    nc = bacc.Bacc("TRN2", target_bir_lowering=False, debug=False,
                   num_devices=N_CORES)

    n_halves = shard // HALF
    xt_e = nc.declare_dram_parameter("xt", [n_halves, 128, K_TILES, HALF], DTR, isOutput=False)
    att_e = nc.declare_dram_parameter("att4", [4, shard], DT, isOutput=False)
    w1_e = nc.declare_dram_parameter("w1", [128, K_TILES, HID_DIM], DTR, isOutput=False)
    w2_e = nc.declare_dram_parameter("w2", [128, H_TILES, OUT_DIM], DTR, isOutput=False)
    b1_e = nc.declare_dram_parameter("b1c", [128, H_TILES], DT, isOutput=False)
    b2_e = nc.declare_dram_parameter("b2c", [OUT_DIM, 2], DT, isOutput=False)
    pos_e = nc.declare_dram_parameter("pos_t", [OUT_DIM, shard], DT, isOutput=True)
    neg_e = nc.declare_dram_parameter("neg_t", [OUT_DIM, shard], DT, isOutput=True)
    net_e = nc.declare_dram_parameter("net_t", [OUT_DIM, shard], DT, isOutput=True)
    bag_e = nc.declare_dram_parameter("bag_t", [OUT_DIM, n_groups], DT, isOutput=True)

    with tile.TileContext(nc) as tc:
        with (
            tc.tile_pool(name="const", bufs=1) as constp,
            tc.tile_pool(name="xtp", bufs=5) as xtp,
            tc.tile_pool(name="hp", bufs=2) as hp,
            tc.tile_pool(name="epp", bufs=2) as epp,
            tc.tile_pool(name="hps", bufs=4, space=bass.MemorySpace.PSUM) as hpsp,
            tc.tile_pool(name="lps", bufs=4, space=bass.MemorySpace.PSUM) as lpsp,
        ):
            w1_sb = constp.tile([128, K_TILES, HID_DIM], DTR)
            nc.sync.dma_start(w1_sb[:], w1_e.ap())
            w2_sb = constp.tile([128, H_TILES, OUT_DIM], DTR)
            nc.sync.dma_start(w2_sb[:], w2_e.ap())
            b1_sb = constp.tile([128, H_TILES], DT)
            nc.sync.dma_start(b1_sb[:], b1_e.ap())
            b2_sb = constp.tile([OUT_DIM, 2], DT)
            nc.sync.dma_start(b2_sb[:], b2_e.ap())
            bag_sb = constp.tile([OUT_DIM, n_groups], DT)
            bagh_sb = constp.tile([OUT_DIM, 2 * n_groups], DT)

            # PE warmup: dummy matmuls flip the HAM clock gate to 8/8
            # while the first xt tiles are still in flight.
            warm_sb = constp.tile([128, HALF], DTR)
            nc.gpsimd.memset(warm_sb[:], 0.0)
            wps = hpsp.tile([128, HALF], DT, tag="hps")
            for _ in range(24):
                nc.tensor.matmul(wps[:], warm_sb[:, 0:128], warm_sb[:],
                                 start=True, stop=True)

            nhalf = GROUP // HALF
            pairs = [list(range(s, min(s + 2, n_groups)))
                     for s in range(0, n_groups, 2)]
            if n_groups >= 4:
                # last two groups run alone: the exit epilogue then covers
                # only 2 halves instead of 4
                pairs = pairs[:-1] + [[n_groups - 2], [n_groups - 1]]
            for gpair in pairs:
                h_byg = {}
                for g in gpair:
                    xt_ts = []
                    for half in range(nhalf):
                        hidx = g * nhalf + half
                        xt_t = xtp.tile([128, K_TILES, HALF], DTR,
                                        tag=f"xt{half}", name=f"xt_{g}_{half}")
                        for kc in range(0, K_TILES, 2):
                            nc.sync.dma_start(xt_t[:, kc:kc + 2, :],
                                              xt_e.ap()[hidx, :, kc:kc + 2, :])
                        xt_ts.append(xt_t)
                    h_all = [[] for _ in range(nhalf)]
                    for hb in range(H_TILES):
                        hpss = [hpsp.tile([128, HALF], DT, tag="hps",
                                          name=f"hps_{g}_{hb}_{i}")
                                for i in range(nhalf)]
                        for k in range(K_TILES):
                            for half in range(nhalf):
                                nc.tensor.matmul(
                                    hpss[half][:],
                                    w1_sb[:, k, hb * 128:(hb + 1) * 128],
                                    xt_ts[half][:, k, :],
                                    start=(k == 0),
                                    stop=(k == K_TILES - 1),
                                )
                        for half in range(nhalf):
                            h_t = hp.tile([128, HALF], DTR,
                                          tag=f"h{hb}x{half}", bufs=3,
                                          name=f"h_{g}_{hb}_{half}")
                            nc.scalar.activation(h_t[:], hpss[half][:], AFT.Relu,
                                                 bias=b1_sb[:, hb:hb + 1])
                            h_all[half].append(h_t)
                    h_byg[g] = h_all
                for g in gpair:
                  h_all = h_byg[g]
                  for half in range(nhalf):
                    hidx = g * nhalf + half
                    # 4 concurrent column-tiled matmuls; partials on PSUM
                    # partition pairs 0/32/64/96, summed via a mixed
                    # PSUM/SBUF chain (ACT copy seeds it)
                    lps = lpsp.tile([128, HALF], DT, tag="lps",
                                    name=f"lps_{g}_{half}")
                    for hb in range(H_TILES):
                        nc.tensor.matmul(
                            lps[32 * hb:32 * hb + 2, :],
                            w2_sb[:, hb, :],
                            h_all[half][hb][:],
                            start=True, stop=True,
                            tile_position=(0, 32 * hb),
                        )
                    c32 = epp.tile([OUT_DIM, HALF], DT, tag="c32")
                    s01 = epp.tile([OUT_DIM, HALF], DT, tag="s01")
                    s02 = epp.tile([OUT_DIM, HALF], DT, tag="s02")
                    lsum = epp.tile([OUT_DIM, HALF], DT, tag="lsum")
                    nc.scalar.copy(c32[:], lps[32:34, :])
                    nc.vector.tensor_add(s01[:], lps[0:2, :], c32[:])
                    nc.vector.scalar_tensor_tensor(
                        s02[:], lps[64:66, :], 1.0, s01[:],
                        mybir.AluOpType.mult, mybir.AluOpType.add)
                    nc.vector.scalar_tensor_tensor(
                        lsum[:], lps[96:98, :], 1.0, s02[:],
                        mybir.AluOpType.mult, mybir.AluOpType.add)
                    relu_p = epp.tile([OUT_DIM, HALF], DT, tag="relu_p")
                    min_n = epp.tile([OUT_DIM, HALF], DT, tag="min_n")
                    nc.scalar.activation(relu_p[:], lsum[:], AFT.Relu,
                                         bias=b2_sb[:, 0:1], scale=1.0)
                    nc.vector.tensor_scalar(min_n[:], lsum[:], b2_sb[:, 0:1], 0.0,
                                            mybir.AluOpType.add,
                                            mybir.AluOpType.min)

                    hsl = slice(hidx * HALF, (hidx + 1) * HALF)
                    att_p = xtp.tile([OUT_DIM, HALF], DT, tag="attp")
                    att_n = xtp.tile([OUT_DIM, HALF], DT, tag="attn")
                    nc.gpsimd.dma_start(att_p[:], att_e.ap()[0:2, hsl])
                    nc.gpsimd.dma_start(att_n[:], att_e.ap()[2:4, hsl])
                    pos_t = epp.tile([OUT_DIM, HALF], DT, tag="pos")
                    neg_t = epp.tile([OUT_DIM, HALF], DT, tag="neg")
                    net_t = epp.tile([OUT_DIM, HALF], DT, tag="net")
                    nc.vector.tensor_mul(pos_t[:], relu_p[:], att_p[:])
                    nc.vector.tensor_mul(neg_t[:], min_n[:], att_n[:])
                    nc.vector.scalar_tensor_tensor(
                        net_t[:], pos_t[:], 1.0, neg_t[:],
                        mybir.AluOpType.mult, mybir.AluOpType.subtract,
                        accum_out=bagh_sb[:, hidx:hidx + 1])
                    nc.gpsimd.dma_start(pos_e.ap()[:, hsl], pos_t[:])
                    nc.gpsimd.dma_start(neg_e.ap()[:, hsl], neg_t[:])
                    nc.gpsimd.dma_start(net_e.ap()[:, hsl], net_t[:])
            nc.vector.tensor_add(bag_sb[:],
                                 bagh_sb.rearrange("p (g two) -> p g two", two=2)[:, :, 0],
                                 bagh_sb.rearrange("p (g two) -> p g two", two=2)[:, :, 1])
            nc.gpsimd.dma_start(bag_e.ap()[:], bag_sb[:])

    nc.compile()
    return nc


_NC_CACHE: dict = {}


def _get_nc(shard: int = SHARD):
    if shard not in _NC_CACHE:
        _NC_CACHE[shard] = build_nc(shard)
    return _NC_CACHE[shard]


def make_in_maps(features, attention, W1, b1, W2, b2, n_cores=N_CORES):
    shard = features.shape[0] // n_cores
    W1 = np.asarray(W1, dtype=np.float32)
    W2 = np.asarray(W2, dtype=np.float32)
    b1 = np.asarray(b1, dtype=np.float32)
    b2 = np.asarray(b2, dtype=np.float32)
    W1 = np.ascontiguousarray(
        W1.reshape(K_TILES, 128, HID_DIM).transpose(1, 0, 2)
    ).astype(ml_dtypes.bfloat16)
    W2 = np.ascontiguousarray(
        W2.reshape(H_TILES, 128, OUT_DIM).transpose(1, 0, 2)
    ).astype(ml_dtypes.bfloat16)
    b1c = np.ascontiguousarray(b1.reshape(H_TILES, 128).T, dtype=np.float32)
    b2c = np.ascontiguousarray(np.stack([b2, -b2], axis=1), dtype=np.float32)
    in_maps = []
    for i in range(n_cores):
        s = i * shard
        fs = features[s:s + shard]
        at = attention[s:s + shard]
        nh = shard // HALF
        xt = np.ascontiguousarray(
            fs.reshape(nh, HALF, K_TILES, 128).transpose(0, 3, 2, 1)
        ).astype(ml_dtypes.bfloat16)
        a0 = at[:, 0]
        a1 = at[:, 1]
        att4 = np.ascontiguousarray(np.stack([a0, a0, -a1, -a1]), dtype=np.float32)
        in_maps.append({
            "xt": xt, "att4": att4, "w1": W1, "w2": W2, "b1c": b1c, "b2c": b2c,
        })
    return in_maps


def assemble_outputs(results, bag_sizes, n_cores=N_CORES):
    pos = np.concatenate([r["pos_t"].T for r in results], axis=0)
    neg = np.concatenate([r["neg_t"].T for r in results], axis=0)
    net = np.concatenate([r["net_t"].T for r in results], axis=0)
    bag_sizes = np.asarray(bag_sizes)
    if bag_sizes.shape[0] == N_BAGS and np.all(bag_sizes == BAG_SIZE):
        bag = np.concatenate([r["bag_t"].T for r in results], axis=0)
    else:
        # Ragged bags: evidence rows are bag-independent; redo only the
        # tiny [N,2]->[B,2] segment sum on the host.
        starts = np.zeros(bag_sizes.shape[0], dtype=np.int64)
        np.cumsum(bag_sizes[:-1], out=starts[1:])
        bag = np.add.reduceat(net, starts, axis=0).astype(np.float32)
    return bag, pos, neg, net


def kernel(features, attention, bag_sizes, W1, b1, W2, b2):
    features = np.asarray(features, dtype=np.float32)
    attention = np.asarray(attention, dtype=np.float32)
    nc = _get_nc(SHARD)
    in_maps = make_in_maps(features, attention, W1, b1, W2, b2)
    res = run_bass_kernel_spmd(nc, in_maps, list(range(N_CORES)))
    return assemble_outputs(res.results, bag_sizes)


# revision 38
# speedup vs baseline: 1.2243x; 1.0052x over previous
"""Trainium2 Bass kernel for nn_AdjunctiveClassifier (segment_reduce).

MLP (1024->512 relu ->2) over 131072 patches + attention-gated evidence +
per-bag segment sum.  Data-parallel over 8 NeuronCores: each core gets a
contiguous shard of 16384 patches (= 16 bags of 1024), weights replicated.

Device layout: everything is kept transposed ([dim, patch]) so that
 - matmul1: h^T[hid,patch] = W1^T-tiles (stationary, natural W1 slices)
            @ X^T-tiles (moving) , accumulated over 8 K-tiles in PSUM
 - matmul2: logits^T[2,patch] = W2-tiles (stationary, natural W2 slices)
            @ h^T (moving), accumulated over 4 hid-tiles
 - epilogue is elementwise on [2, patch] rows, bag sums are free-axis
   reductions over 1024-patch segments.
X^T is produced on the host (free w.r.t. HW exec time); evidence outputs
come back as [2, shard] rows and are transposed back on the host.
The matmul path runs in bfloat16 (1 moving col/cycle on the PE; fp32
would be 4x slower, float32r measured ~1.3 cyc/col due to SBUF
bandwidth); PSUM accumulation and the epilogue stay fp32.  A short
dummy-matmul warmup flips the PE HAM clock gate to full speed while the
first tiles are still in flight.
"""

import sys

import ml_dtypes
import numpy as np

if "/opt/trn_rl_repo" not in sys.path:  # harness safety: concourse import path
    try:
        import concourse  # noqa: F401
    except ImportError:
        sys.path.insert(0, "/opt/trn_rl_repo")

import concourse.bacc as bacc
import concourse.bass as bass
import concourse.mybir as mybir
from concourse import tile
from concourse.bass_utils import run_bass_kernel_spmd

N_PATCHES = 131072
N_BAGS = 128
IN_DIM = 1024
HID_DIM = 512
OUT_DIM = 2
N_CORES = 8
SHARD = N_PATCHES // N_CORES          # 16384 patches per core
BAGS_PER_CORE = N_BAGS // N_CORES     # 16
BAG_SIZE = N_PATCHES // N_BAGS        # 1024
GROUP = BAG_SIZE                      # one bag per group
HALF = 512                            # PSUM-bank / fp32-moving limit
K_TILES = IN_DIM // 128               # 8
H_TILES = HID_DIM // 128              # 4

DT = mybir.dt.float32
DTR = mybir.dt.bfloat16
AFT = mybir.ActivationFunctionType


def build_nc(shard: int = SHARD):
    """Build + compile the per-core Bass graph for a `shard`-patch shard."""
    n_groups = shard // GROUP
    nc = bacc.Bacc("TRN2", target_bir_lowering=False, debug=False,
                   num_devices=N_CORES)

    n_halves = shard // HALF
    xt_e = nc.declare_dram_parameter("xt", [n_halves, 128, K_TILES, HALF], DTR, isOutput=False)
    att_e = nc.declare_dram_parameter("att4", [4, shard], DT, isOutput=False)
    w1_e = nc.declare_dram_parameter("w1", [128, K_TILES, HID_DIM], DTR, isOutput=False)
    w2_e = nc.declare_dram_parameter("w2", [128, H_TILES, OUT_DIM], DTR, isOutput=False)
    b1_e = nc.declare_dram_parameter("b1c", [128, H_TILES], DT, isOutput=False)
    b2_e = nc.declare_dram_parameter("b2c", [OUT_DIM, 2], DT, isOutput=False)
    pos_e = nc.declare_dram_parameter("pos_t", [OUT_DIM, shard], DT, isOutput=True)
    neg_e = nc.declare_dram_parameter("neg_t", [OUT_DIM, shard], DT, isOutput=True)
    net_e = nc.declare_dram_parameter("net_t", [OUT_DIM, shard], DT, isOutput=True)
    bag_e = nc.declare_dram_parameter("bag_t", [OUT_DIM, n_groups], DT, isOutput=True)

    with tile.TileContext(nc) as tc:
        with (
            tc.tile_pool(name="const", bufs=1) as constp,
            tc.tile_pool(name="xtp", bufs=5) as xtp,
            tc.tile_pool(name="hp", bufs=2) as hp,
            tc.tile_pool(name="epp", bufs=2) as epp,
            tc.tile_pool(name="hps", bufs=4, space=bass.MemorySpace.PSUM) as hpsp,
            tc.tile_pool(name="lps", bufs=4, space=bass.MemorySpace.PSUM) as lpsp,
        ):
            w1_sb = constp.tile([128, K_TILES, HID_DIM], DTR)
            nc.sync.dma_start(w1_sb[:], w1_e.ap())
            w2_sb = constp.tile([128, H_TILES, OUT_DIM], DTR)
            nc.sync.dma_start(w2_sb[:], w2_e.ap())
            b1_sb = constp.tile([128, H_TILES], DT)
            nc.sync.dma_start(b1_sb[:], b1_e.ap())
            b2_sb = constp.tile([OUT_DIM, 2], DT)
            nc.sync.dma_start(b2_sb[:], b2_e.ap())
            bag_sb = constp.tile([OUT_DIM, n_groups], DT)
            bagh_sb = constp.tile([OUT_DIM, 2 * n_groups], DT)

            # PE warmup: dummy matmuls flip the HAM clock gate to 8/8
            # while the first xt tiles are still in flight.
            warm_sb = constp.tile([128, HALF], DTR)
            nc.gpsimd.memset(warm_sb[:], 0.0)
            wps = hpsp.tile([128, HALF], DT, tag="hps")
            for _ in range(24):
                nc.tensor.matmul(wps[:], warm_sb[:, 0:128], warm_sb[:],
                                 start=True, stop=True)

            nhalf = GROUP // HALF
            pairs = [list(range(s, min(s + 2, n_groups)))
                     for s in range(0, n_groups, 2)]
            if n_groups >= 4:
                # last two groups run alone: the exit epilogue then covers
                # only 2 halves instead of 4
                pairs = pairs[:-1] + [[n_groups - 2], [n_groups - 1]]
            for gpair in pairs:
                h_byg = {}
                for g in gpair:
                    xt_ts = []
                    for half in range(nhalf):
                        hidx = g * nhalf + half
                        xt_t = xtp.tile([128, K_TILES, HALF], DTR,
                                        tag=f"xt{half}", name=f"xt_{g}_{half}")
                        for kc in range(0, K_TILES, 2):
                            nc.sync.dma_start(xt_t[:, kc:kc + 2, :],
                                              xt_e.ap()[hidx, :, kc:kc + 2, :])
                        xt_ts.append(xt_t)
                    h_all = [[] for _ in range(nhalf)]
                    for hb in range(H_TILES):
                        hpss = [hpsp.tile([128, HALF], DT, tag="hps",
                                          name=f"hps_{g}_{hb}_{i}")
                                for i in range(nhalf)]
                        for k in range(K_TILES):
                            for half in range(nhalf):
                                nc.tensor.matmul(
                                    hpss[half][:],
                                    w1_sb[:, k, hb * 128:(hb + 1) * 128],
                                    xt_ts[half][:, k, :],
                                    start=(k == 0),
                                    stop=(k == K_TILES - 1),
                                )
                        for half in range(nhalf):
                            h_t = hp.tile([128, HALF], DTR,
                                          tag=f"h{hb}x{half}", bufs=3,
                                          name=f"h_{g}_{hb}_{half}")
                            nc.scalar.activation(h_t[:], hpss[half][:], AFT.Relu,
                                                 bias=b1_sb[:, hb:hb + 1])
                            h_all[half].append(h_t)
                    h_byg[g] = h_all
                for g in gpair:
                  h_all = h_byg[g]
                  for half in range(nhalf):
                    hidx = g * nhalf + half
                    # 4 concurrent column-tiled matmuls; partials on PSUM
                    # partition pairs 0/32/64/96, summed via a mixed
                    # PSUM/SBUF chain (ACT copy seeds it)
                    lps = lpsp.tile([128, HALF], DT, tag="lps",
                                    name=f"lps_{g}_{half}")
                    for hb in range(H_TILES):
                        col = 32 * (hb % 2)
                        nc.tensor.matmul(
                            lps[col:col + 2, :],
                            w2_sb[:, hb, :],
                            h_all[half][hb][:],
                            start=(hb < 2),
                            stop=(hb >= 2),
                            tile_position=(0, col),
                        )
                    c32 = epp.tile([OUT_DIM, HALF], DT, tag="c32")
                    lsum = epp.tile([OUT_DIM, HALF], DT, tag="lsum")
                    nc.scalar.copy(c32[:], lps[32:34, :])
                    nc.vector.tensor_add(lsum[:], lps[0:2, :], c32[:])
                    relu_p = epp.tile([OUT_DIM, HALF], DT, tag="relu_p")
                    min_n = epp.tile([OUT_DIM, HALF], DT, tag="min_n")
                    nc.scalar.activation(relu_p[:], lsum[:], AFT.Relu,
                                         bias=b2_sb[:, 0:1], scale=1.0)
                    nc.vector.tensor_scalar(min_n[:], lsum[:], b2_sb[:, 0:1], 0.0,
                                            mybir.AluOpType.add,
                                            mybir.AluOpType.min)

                    hsl = slice(hidx * HALF, (hidx + 1) * HALF)
                    att_p = xtp.tile([OUT_DIM, HALF], DT, tag="attp")
                    att_n = xtp.tile([OUT_DIM, HALF], DT, tag="attn")
                    nc.gpsimd.dma_start(att_p[:], att_e.ap()[0:2, hsl])
                    nc.gpsimd.dma_start(att_n[:], att_e.ap()[2:4, hsl])
                    pos_t = epp.tile([OUT_DIM, HALF], DT, tag="pos")
                    neg_t = epp.tile([OUT_DIM, HALF], DT, tag="neg")
                    net_t = epp.tile([OUT_DIM, HALF], DT, tag="net")
                    nc.vector.tensor_mul(pos_t[:], relu_p[:], att_p[:])
                    nc.vector.tensor_mul(neg_t[:], min_n[:], att_n[:])
                    nc.vector.scalar_tensor_tensor(
                        net_t[:], pos_t[:], 1.0, neg_t[:],
                        mybir.AluOpType.mult, mybir.AluOpType.subtract,
                        accum_out=bagh_sb[:, hidx:hidx + 1])
                    nc.gpsimd.dma_start(pos_e.ap()[:, hsl], pos_t[:])
                    nc.gpsimd.dma_start(neg_e.ap()[:, hsl], neg_t[:])
                    nc.gpsimd.dma_start(net_e.ap()[:, hsl], net_t[:])
            nc.vector.tensor_add(bag_sb[:],
                                 bagh_sb.rearrange("p (g two) -> p g two", two=2)[:, :, 0],
                                 bagh_sb.rearrange("p (g two) -> p g two", two=2)[:, :, 1])
            nc.gpsimd.dma_start(bag_e.ap()[:], bag_sb[:])

    nc.compile()
    return nc


_NC_CACHE: dict = {}


def _get_nc(shard: int = SHARD):
    if shard not in _NC_CACHE:
        _NC_CACHE[shard] = build_nc(shard)
    return _NC_CACHE[shard]


def make_in_maps(features, attention, W1, b1, W2, b2, n_cores=N_CORES):
    shard = features.shape[0] // n_cores
    W1 = np.asarray(W1, dtype=np.float32)
    W2 = np.asarray(W2, dtype=np.float32)
    b1 = np.asarray(b1, dtype=np.float32)
    b2 = np.asarray(b2, dtype=np.float32)
    W1 = np.ascontiguousarray(
        W1.reshape(K_TILES, 128, HID_DIM).transpose(1, 0, 2)
    ).astype(ml_dtypes.bfloat16)
    W2 = np.ascontiguousarray(
        W2.reshape(H_TILES, 128, OUT_DIM).transpose(1, 0, 2)
    ).astype(ml_dtypes.bfloat16)
    b1c = np.ascontiguousarray(b1.reshape(H_TILES, 128).T, dtype=np.float32)
    b2c = np.ascontiguousarray(np.stack([b2, -b2], axis=1), dtype=np.float32)
    in_maps = []
    for i in range(n_cores):
        s = i * shard
        fs = features[s:s + shard]
        at = attention[s:s + shard]
        nh = shard // HALF
        xt = np.ascontiguousarray(
            fs.reshape(nh, HALF, K_TILES, 128).transpose(0, 3, 2, 1)
        ).astype(ml_dtypes.bfloat16)
        a0 = at[:, 0]
        a1 = at[:, 1]
        att4 = np.ascontiguousarray(np.stack([a0, a0, -a1, -a1]), dtype=np.float32)
        in_maps.append({
            "xt": xt, "att4": att4, "w1": W1, "w2": W2, "b1c": b1c, "b2c": b2c,
        })
    return in_maps


def assemble_outputs(results, bag_sizes, n_cores=N_CORES):
    pos = np.concatenate([r["pos_t"].T for r in results], axis=0)
    neg = np.concatenate([r["neg_t"].T for r in results], axis=0)
    net = np.concatenate([r["net_t"].T for r in results], axis=0)
    bag_sizes = np.asarray(bag_sizes)
    if bag_sizes.shape[0] == N_BAGS and np.all(bag_sizes == BAG_SIZE):
        bag = np.concatenate([r["bag_t"].T for r in results], axis=0)
    else:
        # Ragged bags: evidence rows are bag-independent; redo only the
        # tiny [N,2]->[B,2] segment sum on the host.
        starts = np.zeros(bag_sizes.shape[0], dtype=np.int64)
        np.cumsum(bag_sizes[:-1], out=starts[1:])
        bag = np.add.reduceat(net, starts, axis=0).astype(np.float32)
    return bag, pos, neg, net


def kernel(features, attention, bag_sizes, W1, b1, W2, b2):
    features = np.asarray(features, dtype=np.float32)
    attention = np.asarray(attention, dtype=np.float32)
    nc = _get_nc(SHARD)
    in_maps = make_in_maps(features, attention, W1, b1, W2, b2)
    res = run_bass_kernel_spmd(nc, in_maps, list(range(N_CORES)))
    return assemble_outputs(res.results, bag_sizes)
